# revision 7
# baseline (speedup 1.0000x reference)
"""Trainium2 Bass kernel for nn_DistanceLoss (pairwise SmoothL1 distance loss).

reference:
    t[i,j] = sum_d smoothl1(x[i,d] - x[j,d])   (beta=1)  for x in {teacher, student}
    loss = sum |t/mean(t) - s/mean(s)|

identity used (per pair, with d = x_i - x_j):
    smoothl1(d) = 0.5 d^2 - 0.5 relu(|d|-1)^2
    sum_d 0.5 d^2 = 0.5 n_i + 0.5 n_j - G_ij       (Gram decomposition)

The device computes, per core, rows j == k (mod 8) of (-G_ij + corr_ij) for
the upper triangle (corr = -0.5 sum_d relu(|d|-1)^2, accumulated into PSUM by
indicator matmuls). The rank-1 outer-sum 0.5 n_i + 0.5 n_j and the final
mean-normalize/abs-diff reduction are O(N^2) host work in float64.

The nonlinear correction c2 = relu(|x_i - x_j| - 1)^2 is split across engines:
  - fused path (most j's): a custom DVE op (ABSOLUTE_DIFF/sub/max/mul, 4 ALU
    slices) with a hand-written 2X_1PORT uop program (two bf16 elements per
    cycle) and a 2-state subdim FSM so one instruction covers two partition
    tiles (scalar xj switches C0 -> C1 at the subdim boundary).
  - ACT path (largest-fd j's): Abs on the Scalar engine (bias = xj, scale=-1),
    relu(.-1) via tensor_scalar (DVE 4x or Pool), Square via tensor_tensor
    (ACT / DVE 2x / Pool).
  - Pool path (smallest-fd j's): 3-pass tensor_scalar/tensor_tensor chain.
"""

import os
import sys

for _p in ("/opt/trn_rl_repo", "/root/.axon_site/_ro/trn_rl_repo"):
    if _p not in sys.path:
        sys.path.insert(0, _p)

import numpy as np
import ml_dtypes

N = 512
D = 512
NCORES = 8
JB = N // NCORES  # 64 rows of the pair matrix per core
NT = D // 128  # 4 partition tiles of the transposed layout

# work split (see calc in transcript): jl < K -> ACT path; jl >= JB-NPOOL ->
# Pool path; rest -> fused DVE path. Within the ACT path, Square runs on ACT
# for jl < NSQA, on Pool for jl >= K-NSQP, else on DVE; the relu step runs on
# Pool for jl >= K-VP, else on DVE.
K = int(os.environ.get("SL1_K", "9"))
NSQA = int(os.environ.get("SL1_NSQA", "6"))
NSQP = int(os.environ.get("SL1_NSQP", "3"))
NPOOL = int(os.environ.get("SL1_NPOOL", "8"))
VP = int(os.environ.get("SL1_VP", "4"))
PERF = os.environ.get("SL1_PERF", "1") == "1"  # declare 2X_1PORT on custom ops
SUBD = os.environ.get("SL1_SUBD", "1") == "1"  # 2-subdim paired custom op
RUNWAY = int(os.environ.get("SL1_RUN", "2"))
LAG = int(os.environ.get("SL1_LAG", "3"))
COPY_ENG = os.environ.get("SL1_COPY", "act")

_CACHE = {}


def _fd(jl):
    return N - 8 * jl


def _register_custom_ops():
    """Register the smooth-l1 correction ops:
      SL1P_ANT: paired 2-subdim op, out[:,r,:] = relu(|in0[:,r,:] - s_r| - 1)^2
      SL1X_ANT: plain single-tile op,  out = relu(|in0 - s0| - 1)^2
    Both carry a REGULAR program (from the spec compiler) and a hand-written
    2X_1PORT program (4 ALU slices per element, lo in blocks 0-3, hi in 4-7),
    registered via the perf-mode table slots."""
    import copy as _copy

    import concourse.dve_ops as dve_ops
    from concourse.dve_spec import Spec, Src0, C0, Zero, One, maxx, sq, lower, Bin
    from concourse.dve_uop import (
        DveOpSpec,
        UopConfig,
        UopDpConfig,
        AluOp,
        AluInp,
        InpSel,
        OutSel,
        OutPath,
        DelayInp,
        Trigger,
        ENABLE,
    )

    existing = {op.name: op for op in dve_ops.OPS}
    if "SL1P_ANT" in existing and "SL1X_ANT" in existing:
        return existing["SL1P_ANT"], existing["SL1X_ANT"]

    body = sq(maxx(Bin(AluOp.ABSOLUTE_DIFF, Src0, C0) - One, Zero))

    def _ref_plain(in0, in1, s0, s1, imm2):
        x = in0.astype(np.float32)
        return np.square(np.maximum(np.abs(x - s0) - 1.0, 0.0)).astype(np.float32)

    def _ref_pair(in0, in1, s0, s1, imm2):
        x = in0.astype(np.float32)
        out = np.empty_like(x)
        out[:, 0] = np.square(np.maximum(np.abs(x[:, 0] - s0) - 1.0, 0.0))
        out[:, 1] = np.square(np.maximum(np.abs(x[:, 1] - s1) - 1.0, 0.0))
        return out

    base = lower(Spec(body=body, reference=_ref_plain), ver="v3")[0]

    ST, SD, NONE = (
        Trigger.SRC_TENSOR_DONE,
        Trigger.SUB_DIM_DONE,
        Trigger.NONE,
    )

    def _patch(u, const_sel, trig, nxt):
        v = _copy.deepcopy(u)
        for i in range(len(v.inp)):
            if v.inp_enable[i] and v.inp[i] == InpSel.CONST_0:
                v.inp[i] = const_sel
        v.trigger = trig
        v.next_uop = nxt
        return v

    def _mk2x(const_sel, trig, nxt):
        u = UopConfig()
        u.enable_input(InpSel.SRC_0, 1)
        u.enable_input(const_sel, 2)
        u.enable_input(InpSel.ONE_F32, 3)
        u.enable_input(InpSel.ZERO, 4)
        u.enable_input(InpSel.SRC_0_HI, 5)
        P = AluInp.PREV_ALU_OUT
        Dl = (
            AluInp.PREV_DELAY_0,
            AluInp.PREV_DELAY_1,
            AluInp.PREV_DELAY_2,
            AluInp.PREV_DELAY_3,
            AluInp.PREV_DELAY_4,
        )
        dp = u.datapath_config
        # lo element: blocks 0-3; chains: 0=src_lo 1=const 2=one 3=zero 4=src_hi
        dp[0] = (
            UopDpConfig()
            .enable_alu(AluOp.ABSOLUTE_DIFF, Dl[0], Dl[1])
            .pass_through_delay(1, 2, 3, 4)
        )
        dp[1] = (
            UopDpConfig()
            .enable_alu(AluOp.SUBTRACT, P, Dl[2])
            .pass_through_delay(1, 2, 3, 4)
        )
        dp[2] = (
            UopDpConfig()
            .enable_alu(AluOp.MAX, P, Dl[3])
            .pass_through_delay(1, 2, 3, 4)
        )
        dp[3] = (
            UopDpConfig()
            .enable_alu(AluOp.MULTIPLY, P, P)
            .pass_through_delay(1, 2, 3, 4)
        )
        # hi element: blocks 4-7; lo result rides chain 0 from block 4 on
        dp[4] = (
            UopDpConfig()
            .enable_alu(AluOp.ABSOLUTE_DIFF, Dl[4], Dl[1])
            .enable_delay_from_src(DelayInp.PREV_ALU_OUT, 0)
            .pass_through_delay(2, 3)
        )
        dp[5] = (
            UopDpConfig()
            .enable_alu(AluOp.SUBTRACT, P, Dl[2])
            .pass_through_delay(0, 3)
        )
        dp[6] = UopDpConfig().enable_alu(AluOp.MAX, P, Dl[3]).pass_through_delay(0)
        dp[7] = UopDpConfig().enable_alu(AluOp.MULTIPLY, P, P).pass_through_delay(0)
        u.enable_output(OutSel.DELAY_0, OutPath.WR0_LO)
        u.enable_output(OutSel.ALU_OUT, OutPath.WR0_HI)
        u.require_inp0 = ENABLE
        u.trigger = trig
        u.next_uop = nxt
        return u

    def _reg(name, spec, regular, uops_2x, subdim):
        row = dve_ops._CUSTOM_DVE_ROW_BASE + len(dve_ops.OPS)
        dspec = DveOpSpec(
            name=name,
            opcode=row,
            uops=regular,
            uops_2x=uops_2x,
            perf_max=1,
            rd1_en=False,
        )
        for u in regular + uops_2x:
            u.validate("v3")
        op = dve_ops.DveOp(
            name, spec, subdim=subdim, uops_sha={"v3": dspec.sha("v3")}
        )
        dve_ops.OPS.append(op)
        dve_ops._SUB_OPCODE_FOR_NAME[name] = row
        dve_ops.CUSTOM_DVE_SPECS[name] = spec
        dve_ops._COMPILE_CACHE[(name, "v3")] = dspec
        return op

    sl1p = _reg(
        "SL1P_ANT",
        Spec(body=body, reference=_ref_pair),
        [
            _patch(base, InpSel.CONST_0, (ST, SD, NONE), (0, 1, 0)),
            _patch(base, InpSel.CONST_1, (ST, SD, NONE), (0, 1, 0)),
        ],
        [
            _mk2x(InpSel.CONST_0, (ST, SD, NONE), (0, 1, 0)),
            _mk2x(InpSel.CONST_1, (ST, SD, NONE), (0, 1, 0)),
        ],
        subdim=True,
    )
    sl1x = _reg(
        "SL1X_ANT",
        Spec(body=body, reference=_ref_plain),
        [_patch(base, InpSel.CONST_0, (ST, NONE, NONE), (0, 0, 0))],
        [_mk2x(InpSel.CONST_0, (ST, NONE, NONE), (0, 0, 0))],
        subdim=False,
    )
    return sl1p, sl1x


def _path(jl):
    if jl < K:
        return "A"
    if jl >= JB - NPOOL:
        return "P"
    return "F"


def _build_nc(repeat=1):
    import concourse.bacc as bacc
    import concourse.tile as tile
    from concourse import mybir

    sl1p, sl1x = _register_custom_ops()

    dt = mybir.dt
    nc = bacc.Bacc("TRN2", target_bir_lowering=False, debug=False,
                   num_devices=NCORES)

    dram = {}
    for pfx in ("t", "s"):
        dram[pfx + "_xt"] = nc.dram_tensor(pfx + "_xt", [D, N], dt.bfloat16,
                                           kind="ExternalInput").ap()
        dram[pfx + "_xj32"] = nc.dram_tensor(pfx + "_xj32", [D, JB], dt.float32,
                                             kind="ExternalInput").ap()
        dram[pfx + "_out"] = nc.dram_tensor(pfx + "_out", [JB, N], dt.float32,
                                            kind="ExternalOutput").ap()

    with tile.TileContext(nc) as tc:
        import contextlib

        with contextlib.ExitStack() as ctx:
            singles = ctx.enter_context(tc.tile_pool(name="singles", bufs=1))
            qpool = ctx.enter_context(tc.tile_pool(name="qpool", bufs=8))
            apool = ctx.enter_context(tc.tile_pool(name="apool", bufs=4))
            vpool = ctx.enter_context(tc.tile_pool(name="vpool", bufs=4))
            ppool = ctx.enter_context(tc.tile_pool(name="ppool", bufs=3))
            opool = ctx.enter_context(tc.tile_pool(name="opool", bufs=2))
            psp = ctx.enter_context(tc.tile_pool(name="psp", bufs=2, space="PSUM"))

            # sliding -0.5 indicator for the correction reduction matmuls
            zo = singles.tile([128, 128], dt.bfloat16)
            nc.gpsimd.memset(zo, 0.0)
            nc.gpsimd.memset(zo[:, 63:64], -0.5)

            _ord = ("s", "t") if os.environ.get("SL1_SWAP", "") == "1" else ("t", "s")
            _phases = [p for _ in range(repeat) for p in _ord]

            # all input DMAs upfront so the SP DMA queue never blocks a
            # later phase's loads behind an earlier phase's output
            xt_all, xj32_all, negxj_all = {}, {}, {}
            for pfx in _phases:
                # xt pairs: [128, 2, N] tiles so one paired DVE op spans two
                # partition tiles (subrow r holds d-rows 128*(2u+r)+p)
                xt = []
                for u in range(2):
                    x = singles.tile([128, 2, N], dt.bfloat16, tag=f"{pfx}_xt{u}")
                    for r in range(2):
                        t = 2 * u + r
                        nc.sync.dma_start(
                            out=x[:, r, :],
                            in_=dram[pfx + "_xt"][128 * t:128 * (t + 1), :])
                    xt.append(x)
                xj32 = []
                for t in range(NT):
                    p = singles.tile([128, JB], dt.float32, tag=f"{pfx}_xj32{t}")
                    nc.sync.dma_start(
                        out=p, in_=dram[pfx + "_xj32"][128 * t:128 * (t + 1), :])
                    xj32.append(p)
                # -xj in bf16: stationary for the -G matmuls
                negxj = []
                for t in range(NT):
                    nb = singles.tile([128, JB], dt.bfloat16, tag=f"{pfx}_negxj{t}")
                    nc.gpsimd.tensor_scalar(nb, xj32[t], -1.0, None,
                                            mybir.AluOpType.mult)
                    negxj.append(nb)
                xt_all[pfx], xj32_all[pfx], negxj_all[pfx] = xt, xj32, negxj

            accs, outs = {}, {}
            for _pi, pfx in enumerate(_phases):
                xt, xj32, negxj = xt_all[pfx], xj32_all[pfx], negxj_all[pfx]

                def _xts(t, sl=slice(None)):
                    return xt[t // 2][:, t % 2, sl]

                acc = psp.tile([JB, N], dt.float32, tag=f"{pfx}_acc")
                accs[pfx] = acc
                for t in range(NT):
                    nc.tensor.matmul(acc, negxj[t], _xts(t), start=(t == 0),
                                     stop=False)

                # interleave the three paths so every engine has runnable work
                a_js = [j for j in range(JB) if _path(j) == "A"]
                f_js = [j for j in range(JB) if _path(j) == "F"]
                p_js = [j for j in range(JB) if _path(j) == "P"]
                order = []
                ia = if_ = ip = 0
                for _ in range(min(RUNWAY, len(f_js))):
                    order.append(f_js[if_]); if_ += 1
                INF = float("inf")
                while len(order) < JB:
                    ra = ia / len(a_js) if ia < len(a_js) else INF
                    rf = if_ / len(f_js) if if_ < len(f_js) else INF
                    rp = (ip / len(p_js)) * 1.35 if ip < len(p_js) else INF
                    m = min(ra, rf, rp)
                    if ra == m:
                        order.append(a_js[ia]); ia += 1
                    elif rf == m:
                        order.append(f_js[if_]); if_ += 1
                    else:
                        order.append(p_js[ip]); ip += 1

                # software-pipelined emission: each jl's cross-engine chain
                # (produce -> relu -> square -> matmul) is staggered by LAG
                # slots so no in-order engine queue waits at its head
                state = {}
                n_mm = [0]
                total_mm = 4 * JB

                def st_produce(jl):
                    fd = _fd(jl)
                    i0 = N - fd
                    path = _path(jl)
                    q4 = qpool.tile([128, NT, N], dt.bfloat16, tag="q4")
                    state[jl] = {"q4": q4}
                    if path == "F":
                        if SUBD:
                            for u in range(2):
                                bop = nc.vector._custom_dve(
                                    sl1p,
                                    out=q4[:, 2 * u:2 * u + 2, 0:fd],
                                    in0=xt[u][:, :, i0:N],
                                    s0=xj32[2 * u][:, jl:jl + 1],
                                    s1=xj32[2 * u + 1][:, jl:jl + 1])
                                if PERF:
                                    bop.ins.perf_max = 1
                        else:
                            for t in range(NT):
                                bop = nc.vector._custom_dve(
                                    sl1x,
                                    out=q4[:, t, 0:fd],
                                    in0=_xts(t, slice(i0, N)),
                                    s0=xj32[t][:, jl:jl + 1])
                                if PERF:
                                    bop.ins.perf_max = 1
                    elif path == "A":
                        a4 = apool.tile([128, NT, N], dt.bfloat16, tag="a4")
                        for t in range(NT):
                            nc.scalar.activation(a4[:, t, 0:fd],
                                                 _xts(t, slice(i0, N)),
                                                 mybir.ActivationFunctionType.Abs,
                                                 bias=xj32[t][:, jl:jl + 1],
                                                 scale=-1.0)
                        state[jl]["a4"] = a4
                    else:  # "P"
                        p4 = ppool.tile([128, NT, N], dt.bfloat16, tag="p4")
                        for t in range(NT):
                            nc.gpsimd.tensor_scalar(p4[:, t, 0:fd],
                                                    _xts(t, slice(i0, N)),
                                                    xj32[t][:, jl:jl + 1], 0.0,
                                                    mybir.AluOpType.subtract,
                                                    mybir.AluOpType.abs_max)
                        v4 = vpool.tile([128, NT, N], dt.bfloat16, tag="v4")
                        nc.gpsimd.tensor_scalar(v4[:, :, 0:fd], p4[:, :, 0:fd],
                                                1.0, 0.0,
                                                mybir.AluOpType.subtract,
                                                mybir.AluOpType.max)
                        nc.gpsimd.tensor_tensor(q4[:, :, 0:fd], v4[:, :, 0:fd],
                                                v4[:, :, 0:fd],
                                                mybir.AluOpType.mult)

                def st_relu(jl):
                    if _path(jl) != "A":
                        return
                    fd = _fd(jl)
                    v4 = vpool.tile([128, NT, N], dt.bfloat16, tag="v4")
                    a4 = state[jl]["a4"]
                    veng = nc.gpsimd if jl >= K - VP else nc.vector
                    veng.tensor_scalar(v4[:, :, 0:fd], a4[:, :, 0:fd],
                                       1.0, 0.0, mybir.AluOpType.subtract,
                                       mybir.AluOpType.max)
                    state[jl]["v4"] = v4

                def st_square(jl):
                    if _path(jl) != "A":
                        return
                    fd = _fd(jl)
                    v4 = state[jl]["v4"]
                    q4 = state[jl]["q4"]
                    if jl < NSQA:
                        nc.scalar.activation(q4[:, :, 0:fd], v4[:, :, 0:fd],
                                             mybir.ActivationFunctionType.Square,
                                             bias=0.0, scale=1.0)
                    else:
                        sqeng = nc.gpsimd if jl >= K - NSQP else nc.vector
                        sqeng.tensor_tensor(q4[:, :, 0:fd], v4[:, :, 0:fd],
                                            v4[:, :, 0:fd], mybir.AluOpType.mult)

                def st_matmul(jl):
                    fd = _fd(jl)
                    i0 = N - fd
                    q4 = state[jl]["q4"]
                    for t in range(NT):
                        n_mm[0] += 1
                        nc.tensor.matmul(acc[:, i0:N], zo[:, 63 - jl:127 - jl],
                                         q4[:, t, 0:fd],
                                         start=False, stop=(n_mm[0] == total_mm))
                    del state[jl]

                stages = (st_produce, st_relu, st_square, st_matmul)
                lags = (0, 1, 2, LAG)
                for slot in range(len(order) + LAG):
                    for stage, lag in zip(stages, lags):
                        i = slot - lag
                        if 0 <= i < len(order):
                            stage(order[i])

            # copies + output DMAs at the very end: nothing queues behind them
            for pfx in _phases:
                out_sb = opool.tile([JB, N], dt.float32, tag=f"{pfx}_out")
                if COPY_ENG == "pool":
                    nc.gpsimd.tensor_copy(out_sb, accs[pfx])
                elif COPY_ENG == "dve":
                    nc.vector.tensor_copy(out_sb, accs[pfx])
                else:
                    nc.scalar.copy(out_sb, accs[pfx])
                nc.sync.dma_start(out=dram[pfx + "_out"], in_=out_sb)

    nc.finalize()
    return nc


def _get_nc(repeat=1):
    key = ("nc", repeat)
    if key not in _CACHE:
        _CACHE[key] = _build_nc(repeat=repeat)
    return _CACHE[key]


def _prep_inputs(teacher, student):
    in_maps = []
    prepped = {}
    for pfx, x in (("t", teacher), ("s", student)):
        xb = np.asarray(x, np.float32).astype(ml_dtypes.bfloat16)   # [N, D] bf16
        xtb = np.ascontiguousarray(xb.T)                            # [D, N] bf16
        xtb32 = xtb.astype(np.float32)  # bf16-rounded values, exact in fp32
        prepped[pfx] = (xtb, xtb32)
    for k in range(NCORES):
        m = {}
        for pfx in ("t", "s"):
            xtb, xtb32 = prepped[pfx]
            m[pfx + "_xt"] = xtb
            m[pfx + "_xj32"] = np.ascontiguousarray(xtb32[:, k::8])
        in_maps.append(m)
    return in_maps


def _assemble(blocks, n):
    """blocks: list of [JB, N] device rows (-G + corr) per core; n: [N] fp64
    squared-norm vector. Returns the full symmetric pair-sum matrix [N, N]."""
    U = np.zeros((N, N), np.float64)
    for k in range(NCORES):
        b = blocks[k].astype(np.float64)
        for jl in range(JB):
            j = 8 * jl + k
            U[j, j + 1:] = b[jl, j + 1:]
    T = U + U.T
    M = 0.5 * (n[:, None] + n[None, :])
    np.fill_diagonal(M, 0.0)
    T += M
    np.fill_diagonal(T, 0.0)
    return T


def run_device(teacher, student, **kwargs):
    """Run the device part; returns (T, S) full pair-sum matrices and results."""
    from concourse.bass_utils import run_bass_kernel_spmd

    nc = _get_nc()
    in_maps = _prep_inputs(teacher, student)
    res = run_bass_kernel_spmd(nc, in_maps, core_ids=list(range(NCORES)), **kwargs)
    ns = {}
    for pfx, x in (("t", teacher), ("s", student)):
        xb32 = np.asarray(x, np.float32).astype(ml_dtypes.bfloat16).astype(np.float64)
        ns[pfx] = np.square(xb32).sum(axis=1)
    T = _assemble([res.results[k]["t_out"] for k in range(NCORES)], ns["t"])
    S = _assemble([res.results[k]["s_out"] for k in range(NCORES)], ns["s"])
    return T, S, res


def kernel(teacher, student):
    teacher = np.asarray(teacher)
    student = np.asarray(student)
    T, S, _ = run_device(teacher, student)
    out = np.abs(T / T.mean() - S / S.mean()).sum()
    return np.float32(out)


if __name__ == "__main__":
    rng = np.random.default_rng(0)
    t = rng.standard_normal((N, D)).astype(np.float32)
    s = rng.standard_normal((N, D)).astype(np.float32)
    print(kernel(t, s))


# revision 9
# speedup vs baseline: 1.2448x; 1.2448x over previous
"""Trainium2 Bass kernel for nn_DistanceLoss (pairwise SmoothL1 distance loss).

reference:
    t[i,j] = sum_d smoothl1(x[i,d] - x[j,d])   (beta=1)  for x in {teacher, student}
    loss = sum |t/mean(t) - s/mean(s)|

identity used (per pair, with d = x_i - x_j):
    smoothl1(d) = 0.5 d^2 - 0.5 relu(|d|-1)^2
    sum_d 0.5 d^2 = 0.5 n_i + 0.5 n_j - G_ij       (Gram decomposition)

The device computes, per core, rows j == k (mod 8) of (-G_ij + corr_ij) for
the upper triangle (corr = -0.5 sum_d relu(|d|-1)^2, accumulated into PSUM by
indicator matmuls). The rank-1 outer-sum 0.5 n_i + 0.5 n_j and the final
mean-normalize/abs-diff reduction are O(N^2) host work in float64.

The nonlinear correction c2 = relu(|x_i - x_j| - 1)^2 is split across engines:
  - fused path (most j's): a custom DVE op (ABSOLUTE_DIFF/sub/max/mul, 4 ALU
    slices) with a hand-written 2X_1PORT uop program (two bf16 elements per
    cycle) and a 2-state subdim FSM so one instruction covers two partition
    tiles (scalar xj switches C0 -> C1 at the subdim boundary).
  - ACT path (largest-fd j's): Abs on the Scalar engine (bias = xj, scale=-1),
    relu(.-1) via tensor_scalar (DVE 4x or Pool), Square via tensor_tensor
    (ACT / DVE 2x / Pool).
  - Pool path (smallest-fd j's): 3-pass tensor_scalar/tensor_tensor chain.
"""

import os
import sys

for _p in ("/opt/trn_rl_repo", "/root/.axon_site/_ro/trn_rl_repo"):
    if _p not in sys.path:
        sys.path.insert(0, _p)

import numpy as np
import ml_dtypes

N = 512
D = 512
NCORES = 8
JB = N // NCORES  # 64 rows of the pair matrix per core
NT = D // 128  # 4 partition tiles of the transposed layout

# work split (see calc in transcript): jl < K -> ACT path; jl >= JB-NPOOL ->
# Pool path; rest -> fused DVE path. Within the ACT path, Square runs on ACT
# for jl < NSQA, on Pool for jl >= K-NSQP, else on DVE; the relu step runs on
# Pool for jl >= K-VP, else on DVE.
K = int(os.environ.get("SL1_K", "8"))
NSQA = int(os.environ.get("SL1_NSQA", "8"))
NSQP = int(os.environ.get("SL1_NSQP", "0"))
NPOOL = int(os.environ.get("SL1_NPOOL", "16"))
VP = int(os.environ.get("SL1_VP", "0"))
PERF = os.environ.get("SL1_PERF", "1") == "1"  # declare 2X_1PORT on custom ops
SUBD = os.environ.get("SL1_SUBD", "1") == "1"  # 2-subdim paired custom op
RUNWAY = int(os.environ.get("SL1_RUN", "2"))
LAG = int(os.environ.get("SL1_LAG", "3"))
COPY_ENG = os.environ.get("SL1_COPY", "act")

_CACHE = {}


def _fd(jl):
    return N - 8 * jl


def _register_custom_ops():
    """Register the smooth-l1 correction ops:
      SL1P_ANT: paired 2-subdim op, out[:,r,:] = relu(|in0[:,r,:] - s_r| - 1)^2
      SL1X_ANT: plain single-tile op,  out = relu(|in0 - s0| - 1)^2
    Both carry a REGULAR program (from the spec compiler) and a hand-written
    2X_1PORT program (4 ALU slices per element, lo in blocks 0-3, hi in 4-7),
    registered via the perf-mode table slots."""
    import copy as _copy

    import concourse.dve_ops as dve_ops
    from concourse.dve_spec import Spec, Src0, C0, Zero, One, maxx, sq, lower, Bin
    from concourse.dve_uop import (
        DveOpSpec,
        UopConfig,
        UopDpConfig,
        AluOp,
        AluInp,
        InpSel,
        OutSel,
        OutPath,
        DelayInp,
        Trigger,
        ENABLE,
    )

    existing = {op.name: op for op in dve_ops.OPS}
    if "SL1P_ANT" in existing and "SL1X_ANT" in existing:
        return existing["SL1P_ANT"], existing["SL1X_ANT"]

    body = sq(maxx(Bin(AluOp.ABSOLUTE_DIFF, Src0, C0) - One, Zero))

    def _ref_plain(in0, in1, s0, s1, imm2):
        x = in0.astype(np.float32)
        return np.square(np.maximum(np.abs(x - s0) - 1.0, 0.0)).astype(np.float32)

    def _ref_pair(in0, in1, s0, s1, imm2):
        x = in0.astype(np.float32)
        out = np.empty_like(x)
        out[:, 0] = np.square(np.maximum(np.abs(x[:, 0] - s0) - 1.0, 0.0))
        out[:, 1] = np.square(np.maximum(np.abs(x[:, 1] - s1) - 1.0, 0.0))
        return out

    base = lower(Spec(body=body, reference=_ref_plain), ver="v3")[0]

    ST, SD, NONE = (
        Trigger.SRC_TENSOR_DONE,
        Trigger.SUB_DIM_DONE,
        Trigger.NONE,
    )

    def _patch(u, const_sel, trig, nxt):
        v = _copy.deepcopy(u)
        for i in range(len(v.inp)):
            if v.inp_enable[i] and v.inp[i] == InpSel.CONST_0:
                v.inp[i] = const_sel
        v.trigger = trig
        v.next_uop = nxt
        return v

    def _mk2x(const_sel, trig, nxt):
        u = UopConfig()
        u.enable_input(InpSel.SRC_0, 1)
        u.enable_input(const_sel, 2)
        u.enable_input(InpSel.ONE_F32, 3)
        u.enable_input(InpSel.ZERO, 4)
        u.enable_input(InpSel.SRC_0_HI, 5)
        P = AluInp.PREV_ALU_OUT
        Dl = (
            AluInp.PREV_DELAY_0,
            AluInp.PREV_DELAY_1,
            AluInp.PREV_DELAY_2,
            AluInp.PREV_DELAY_3,
            AluInp.PREV_DELAY_4,
        )
        dp = u.datapath_config
        # lo element: blocks 0-3; chains: 0=src_lo 1=const 2=one 3=zero 4=src_hi
        dp[0] = (
            UopDpConfig()
            .enable_alu(AluOp.ABSOLUTE_DIFF, Dl[0], Dl[1])
            .pass_through_delay(1, 2, 3, 4)
        )
        dp[1] = (
            UopDpConfig()
            .enable_alu(AluOp.SUBTRACT, P, Dl[2])
            .pass_through_delay(1, 2, 3, 4)
        )
        dp[2] = (
            UopDpConfig()
            .enable_alu(AluOp.MAX, P, Dl[3])
            .pass_through_delay(1, 2, 3, 4)
        )
        dp[3] = (
            UopDpConfig()
            .enable_alu(AluOp.MULTIPLY, P, P)
            .pass_through_delay(1, 2, 3, 4)
        )
        # hi element: blocks 4-7; lo result rides chain 0 from block 4 on
        dp[4] = (
            UopDpConfig()
            .enable_alu(AluOp.ABSOLUTE_DIFF, Dl[4], Dl[1])
            .enable_delay_from_src(DelayInp.PREV_ALU_OUT, 0)
            .pass_through_delay(2, 3)
        )
        dp[5] = (
            UopDpConfig()
            .enable_alu(AluOp.SUBTRACT, P, Dl[2])
            .pass_through_delay(0, 3)
        )
        dp[6] = UopDpConfig().enable_alu(AluOp.MAX, P, Dl[3]).pass_through_delay(0)
        dp[7] = UopDpConfig().enable_alu(AluOp.MULTIPLY, P, P).pass_through_delay(0)
        u.enable_output(OutSel.DELAY_0, OutPath.WR0_LO)
        u.enable_output(OutSel.ALU_OUT, OutPath.WR0_HI)
        u.require_inp0 = ENABLE
        u.trigger = trig
        u.next_uop = nxt
        return u

    def _reg(name, spec, regular, uops_2x, subdim):
        row = dve_ops._CUSTOM_DVE_ROW_BASE + len(dve_ops.OPS)
        dspec = DveOpSpec(
            name=name,
            opcode=row,
            uops=regular,
            uops_2x=uops_2x,
            perf_max=1,
            rd1_en=False,
        )
        for u in regular + uops_2x:
            u.validate("v3")
        op = dve_ops.DveOp(
            name, spec, subdim=subdim, uops_sha={"v3": dspec.sha("v3")}
        )
        dve_ops.OPS.append(op)
        dve_ops._SUB_OPCODE_FOR_NAME[name] = row
        dve_ops.CUSTOM_DVE_SPECS[name] = spec
        dve_ops._COMPILE_CACHE[(name, "v3")] = dspec
        return op

    sl1p = _reg(
        "SL1P_ANT",
        Spec(body=body, reference=_ref_pair),
        [
            _patch(base, InpSel.CONST_0, (ST, SD, NONE), (0, 1, 0)),
            _patch(base, InpSel.CONST_1, (ST, SD, NONE), (0, 1, 0)),
        ],
        [
            _mk2x(InpSel.CONST_0, (ST, SD, NONE), (0, 1, 0)),
            _mk2x(InpSel.CONST_1, (ST, SD, NONE), (0, 1, 0)),
        ],
        subdim=True,
    )
    sl1x = _reg(
        "SL1X_ANT",
        Spec(body=body, reference=_ref_plain),
        [_patch(base, InpSel.CONST_0, (ST, NONE, NONE), (0, 0, 0))],
        [_mk2x(InpSel.CONST_0, (ST, NONE, NONE), (0, 0, 0))],
        subdim=False,
    )
    return sl1p, sl1x


def _path(jl):
    if jl < K:
        return "A"
    if jl >= JB - NPOOL:
        return "P"
    return "F"


def _build_nc(repeat=1):
    import concourse.bacc as bacc
    import concourse.tile as tile
    from concourse import mybir

    sl1p, sl1x = _register_custom_ops()

    dt = mybir.dt
    nc = bacc.Bacc("TRN2", target_bir_lowering=False, debug=False,
                   num_devices=NCORES)

    dram = {}
    for pfx in ("t", "s"):
        dram[pfx + "_xt"] = nc.dram_tensor(pfx + "_xt", [D, N], dt.bfloat16,
                                           kind="ExternalInput").ap()
        dram[pfx + "_xj32"] = nc.dram_tensor(pfx + "_xj32", [D, JB], dt.float32,
                                             kind="ExternalInput").ap()
        dram[pfx + "_out"] = nc.dram_tensor(pfx + "_out", [JB, N], dt.float32,
                                            kind="ExternalOutput").ap()

    with tile.TileContext(nc) as tc:
        import contextlib

        with contextlib.ExitStack() as ctx:
            singles = ctx.enter_context(tc.tile_pool(name="singles", bufs=1))
            qpool = ctx.enter_context(tc.tile_pool(name="qpool", bufs=8))
            apool = ctx.enter_context(tc.tile_pool(name="apool", bufs=4))
            vpool = ctx.enter_context(tc.tile_pool(name="vpool", bufs=4))
            ppool = ctx.enter_context(tc.tile_pool(name="ppool", bufs=3))
            opool = ctx.enter_context(tc.tile_pool(name="opool", bufs=2))
            psp = ctx.enter_context(tc.tile_pool(name="psp", bufs=2, space="PSUM"))

            # sliding -0.5 indicator for the correction reduction matmuls
            zo = singles.tile([128, 128], dt.bfloat16)
            nc.gpsimd.memset(zo, 0.0)
            nc.gpsimd.memset(zo[:, 63:64], -0.5)

            _ord = ("s", "t") if os.environ.get("SL1_SWAP", "") == "1" else ("t", "s")
            _phases = [p for _ in range(repeat) for p in _ord]

            # all input DMAs upfront so the SP DMA queue never blocks a
            # later phase's loads behind an earlier phase's output
            xt_all, xj32_all, negxj_all = {}, {}, {}
            for pfx in _phases:
                # xt pairs: [128, 2, N] tiles so one paired DVE op spans two
                # partition tiles (subrow r holds d-rows 128*(2u+r)+p)
                xt = []
                for u in range(2):
                    x = singles.tile([128, 2, N], dt.bfloat16, tag=f"{pfx}_xt{u}")
                    for r in range(2):
                        t = 2 * u + r
                        nc.sync.dma_start(
                            out=x[:, r, :],
                            in_=dram[pfx + "_xt"][128 * t:128 * (t + 1), :])
                    xt.append(x)
                xj32 = []
                for t in range(NT):
                    p = singles.tile([128, JB], dt.float32, tag=f"{pfx}_xj32{t}")
                    nc.sync.dma_start(
                        out=p, in_=dram[pfx + "_xj32"][128 * t:128 * (t + 1), :])
                    xj32.append(p)
                # -xj in bf16: stationary for the -G matmuls
                negxj = []
                for t in range(NT):
                    nb = singles.tile([128, JB], dt.bfloat16, tag=f"{pfx}_negxj{t}")
                    nc.gpsimd.tensor_scalar(nb, xj32[t], -1.0, None,
                                            mybir.AluOpType.mult)
                    negxj.append(nb)
                xt_all[pfx], xj32_all[pfx], negxj_all[pfx] = xt, xj32, negxj

            accs, outs = {}, {}
            for _pi, pfx in enumerate(_phases):
                xt, xj32, negxj = xt_all[pfx], xj32_all[pfx], negxj_all[pfx]

                def _xts(t, sl=slice(None)):
                    return xt[t // 2][:, t % 2, sl]

                acc = psp.tile([JB, N], dt.float32, tag=f"{pfx}_acc")
                accs[pfx] = acc
                for t in range(NT):
                    nc.tensor.matmul(acc, negxj[t], _xts(t), start=(t == 0),
                                     stop=False)

                # interleave the three paths so every engine has runnable work
                a_js = [j for j in range(JB) if _path(j) == "A"]
                f_js = [j for j in range(JB) if _path(j) == "F"]
                p_js = [j for j in range(JB) if _path(j) == "P"]
                order = []
                ia = if_ = ip = 0
                for _ in range(min(RUNWAY, len(f_js))):
                    order.append(f_js[if_]); if_ += 1
                INF = float("inf")
                while len(order) < JB:
                    ra = ia / len(a_js) if ia < len(a_js) else INF
                    rf = if_ / len(f_js) if if_ < len(f_js) else INF
                    rp = ip / len(p_js) if ip < len(p_js) else INF
                    m = min(ra, rf, rp)
                    if ra == m:
                        order.append(a_js[ia]); ia += 1
                    elif rf == m:
                        order.append(f_js[if_]); if_ += 1
                    else:
                        order.append(p_js[ip]); ip += 1

                # software-pipelined emission: each jl's cross-engine chain
                # (produce -> relu -> square -> matmul) is staggered by LAG
                # slots so no in-order engine queue waits at its head
                state = {}
                n_mm = [0]
                total_mm = 4 * JB

                def st_produce(jl):
                    fd = _fd(jl)
                    i0 = N - fd
                    path = _path(jl)
                    q4 = qpool.tile([128, NT, N], dt.bfloat16, tag="q4")
                    state[jl] = {"q4": q4}
                    if path == "F":
                        if SUBD:
                            for u in range(2):
                                bop = nc.vector._custom_dve(
                                    sl1p,
                                    out=q4[:, 2 * u:2 * u + 2, 0:fd],
                                    in0=xt[u][:, :, i0:N],
                                    s0=xj32[2 * u][:, jl:jl + 1],
                                    s1=xj32[2 * u + 1][:, jl:jl + 1])
                                if PERF:
                                    bop.ins.perf_max = 1
                        else:
                            for t in range(NT):
                                bop = nc.vector._custom_dve(
                                    sl1x,
                                    out=q4[:, t, 0:fd],
                                    in0=_xts(t, slice(i0, N)),
                                    s0=xj32[t][:, jl:jl + 1])
                                if PERF:
                                    bop.ins.perf_max = 1
                    elif path == "A":
                        a4 = apool.tile([128, NT, N], dt.bfloat16, tag="a4")
                        for t in range(NT):
                            nc.scalar.activation(a4[:, t, 0:fd],
                                                 _xts(t, slice(i0, N)),
                                                 mybir.ActivationFunctionType.Abs,
                                                 bias=xj32[t][:, jl:jl + 1],
                                                 scale=-1.0)
                        state[jl]["a4"] = a4
                    else:  # "P"
                        p4 = ppool.tile([128, NT, N], dt.bfloat16, tag="p4")
                        for t in range(NT):
                            nc.gpsimd.tensor_scalar(p4[:, t, 0:fd],
                                                    _xts(t, slice(i0, N)),
                                                    xj32[t][:, jl:jl + 1], 0.0,
                                                    mybir.AluOpType.subtract,
                                                    mybir.AluOpType.abs_max)
                        v4 = vpool.tile([128, NT, N], dt.bfloat16, tag="v4")
                        nc.gpsimd.tensor_scalar(v4[:, :, 0:fd], p4[:, :, 0:fd],
                                                1.0, 0.0,
                                                mybir.AluOpType.subtract,
                                                mybir.AluOpType.max)
                        nc.gpsimd.tensor_tensor(q4[:, :, 0:fd], v4[:, :, 0:fd],
                                                v4[:, :, 0:fd],
                                                mybir.AluOpType.mult)

                def st_relu(jl):
                    if _path(jl) != "A":
                        return
                    fd = _fd(jl)
                    v4 = vpool.tile([128, NT, N], dt.bfloat16, tag="v4")
                    a4 = state[jl]["a4"]
                    veng = nc.gpsimd if jl >= K - VP else nc.vector
                    veng.tensor_scalar(v4[:, :, 0:fd], a4[:, :, 0:fd],
                                       1.0, 0.0, mybir.AluOpType.subtract,
                                       mybir.AluOpType.max)
                    state[jl]["v4"] = v4

                def st_square(jl):
                    if _path(jl) != "A":
                        return
                    fd = _fd(jl)
                    v4 = state[jl]["v4"]
                    q4 = state[jl]["q4"]
                    if jl < NSQA:
                        nc.scalar.activation(q4[:, :, 0:fd], v4[:, :, 0:fd],
                                             mybir.ActivationFunctionType.Square,
                                             bias=0.0, scale=1.0)
                    else:
                        sqeng = nc.gpsimd if jl >= K - NSQP else nc.vector
                        sqeng.tensor_tensor(q4[:, :, 0:fd], v4[:, :, 0:fd],
                                            v4[:, :, 0:fd], mybir.AluOpType.mult)

                def st_matmul(jl):
                    fd = _fd(jl)
                    i0 = N - fd
                    q4 = state[jl]["q4"]
                    for t in range(NT):
                        n_mm[0] += 1
                        nc.tensor.matmul(acc[:, i0:N], zo[:, 63 - jl:127 - jl],
                                         q4[:, t, 0:fd],
                                         start=False, stop=(n_mm[0] == total_mm))
                    del state[jl]

                stages = (st_produce, st_relu, st_square, st_matmul)
                lags = (0, 1, 2, LAG)
                for slot in range(len(order) + LAG):
                    for stage, lag in zip(stages, lags):
                        i = slot - lag
                        if 0 <= i < len(order):
                            stage(order[i])

            # copies + output DMAs at the very end: nothing queues behind them
            for pfx in _phases:
                out_sb = opool.tile([JB, N], dt.float32, tag=f"{pfx}_out")
                if COPY_ENG == "pool":
                    nc.gpsimd.tensor_copy(out_sb, accs[pfx])
                elif COPY_ENG == "dve":
                    nc.vector.tensor_copy(out_sb, accs[pfx])
                else:
                    nc.scalar.copy(out_sb, accs[pfx])
                nc.sync.dma_start(out=dram[pfx + "_out"], in_=out_sb)

    nc.finalize()
    return nc


def _get_nc(repeat=1):
    key = ("nc", repeat)
    if key not in _CACHE:
        _CACHE[key] = _build_nc(repeat=repeat)
    return _CACHE[key]


def _prep_inputs(teacher, student):
    in_maps = []
    prepped = {}
    for pfx, x in (("t", teacher), ("s", student)):
        xb = np.asarray(x, np.float32).astype(ml_dtypes.bfloat16)   # [N, D] bf16
        xtb = np.ascontiguousarray(xb.T)                            # [D, N] bf16
        xtb32 = xtb.astype(np.float32)  # bf16-rounded values, exact in fp32
        prepped[pfx] = (xtb, xtb32)
    for k in range(NCORES):
        m = {}
        for pfx in ("t", "s"):
            xtb, xtb32 = prepped[pfx]
            m[pfx + "_xt"] = xtb
            m[pfx + "_xj32"] = np.ascontiguousarray(xtb32[:, k::8])
        in_maps.append(m)
    return in_maps


def _assemble(blocks, n):
    """blocks: list of [JB, N] device rows (-G + corr) per core; n: [N] fp64
    squared-norm vector. Returns the full symmetric pair-sum matrix [N, N]."""
    U = np.zeros((N, N), np.float64)
    for k in range(NCORES):
        b = blocks[k].astype(np.float64)
        for jl in range(JB):
            j = 8 * jl + k
            U[j, j + 1:] = b[jl, j + 1:]
    T = U + U.T
    M = 0.5 * (n[:, None] + n[None, :])
    np.fill_diagonal(M, 0.0)
    T += M
    np.fill_diagonal(T, 0.0)
    return T


def run_device(teacher, student, **kwargs):
    """Run the device part; returns (T, S) full pair-sum matrices and results."""
    from concourse.bass_utils import run_bass_kernel_spmd

    nc = _get_nc()
    in_maps = _prep_inputs(teacher, student)
    res = run_bass_kernel_spmd(nc, in_maps, core_ids=list(range(NCORES)), **kwargs)
    ns = {}
    for pfx, x in (("t", teacher), ("s", student)):
        xb32 = np.asarray(x, np.float32).astype(ml_dtypes.bfloat16).astype(np.float64)
        ns[pfx] = np.square(xb32).sum(axis=1)
    T = _assemble([res.results[k]["t_out"] for k in range(NCORES)], ns["t"])
    S = _assemble([res.results[k]["s_out"] for k in range(NCORES)], ns["s"])
    return T, S, res


def kernel(teacher, student):
    teacher = np.asarray(teacher)
    student = np.asarray(student)
    T, S, _ = run_device(teacher, student)
    out = np.abs(T / T.mean() - S / S.mean()).sum()
    return np.float32(out)


if __name__ == "__main__":
    rng = np.random.default_rng(0)
    t = rng.standard_normal((N, D)).astype(np.float32)
    s = rng.standard_normal((N, D)).astype(np.float32)
    print(kernel(t, s))


# revision 11
# speedup vs baseline: 1.3015x; 1.0456x over previous
"""Trainium2 Bass kernel for nn_DistanceLoss (pairwise SmoothL1 distance loss).

reference:
    t[i,j] = sum_d smoothl1(x[i,d] - x[j,d])   (beta=1)  for x in {teacher, student}
    loss = sum |t/mean(t) - s/mean(s)|

identity used (per pair, with d = x_i - x_j):
    smoothl1(d) = 0.5 d^2 - 0.5 relu(|d|-1)^2
    sum_d 0.5 d^2 = 0.5 n_i + 0.5 n_j - G_ij       (Gram decomposition)

The device computes, per core, rows j == k (mod 8) of (-G_ij + corr_ij) for
the upper triangle (corr = -0.5 sum_d relu(|d|-1)^2, accumulated into PSUM by
indicator matmuls). The rank-1 outer-sum 0.5 n_i + 0.5 n_j and the final
mean-normalize/abs-diff reduction are O(N^2) host work in float64.

The nonlinear correction c2 = relu(|x_i - x_j| - 1)^2 is split across engines:
  - fused path (most j's): a custom DVE op (ABSOLUTE_DIFF/sub/max/mul, 4 ALU
    slices) with a hand-written 2X_1PORT uop program (two bf16 elements per
    cycle) and a 2-state subdim FSM so one instruction covers two partition
    tiles (scalar xj switches C0 -> C1 at the subdim boundary).
  - ACT path (largest-fd j's): Abs on the Scalar engine (bias = xj, scale=-1),
    relu(.-1) via tensor_scalar (DVE 4x or Pool), Square via tensor_tensor
    (ACT / DVE 2x / Pool).
  - Pool path (smallest-fd j's): 3-pass tensor_scalar/tensor_tensor chain.
"""

import os
import sys

for _p in ("/opt/trn_rl_repo", "/root/.axon_site/_ro/trn_rl_repo"):
    if _p not in sys.path:
        sys.path.insert(0, _p)

import numpy as np
import ml_dtypes

N = 512
D = 512
NCORES = 8
JB = N // NCORES  # 64 rows of the pair matrix per core
NT = D // 128  # 4 partition tiles of the transposed layout

# work split (see calc in transcript): jl < K -> ACT path; jl >= JB-NPOOL ->
# Pool path; rest -> fused DVE path. Within the ACT path, Square runs on ACT
# for jl < NSQA, on Pool for jl >= K-NSQP, else on DVE; the relu step runs on
# Pool for jl >= K-VP, else on DVE.
K = int(os.environ.get("SL1_K", "8"))
NSQA = int(os.environ.get("SL1_NSQA", "8"))
NSQP = int(os.environ.get("SL1_NSQP", "0"))
NPOOL = int(os.environ.get("SL1_NPOOL", "16"))
VP = int(os.environ.get("SL1_VP", "0"))
PERF = os.environ.get("SL1_PERF", "1") == "1"  # declare 2X_1PORT on custom ops
SUBD = os.environ.get("SL1_SUBD", "1") == "1"  # 2-subdim paired custom op
RUNWAY = int(os.environ.get("SL1_RUN", "2"))
LAG = int(os.environ.get("SL1_LAG", "3"))
COPY_ENG = os.environ.get("SL1_COPY", "act")

_CACHE = {}


def _fd(jl):
    return N - 8 * jl


def _register_custom_ops():
    """Register the smooth-l1 correction ops:
      SL1P_ANT: paired 2-subdim op, out[:,r,:] = relu(|in0[:,r,:] - s_r| - 1)^2
      SL1X_ANT: plain single-tile op,  out = relu(|in0 - s0| - 1)^2
    Both carry a REGULAR program (from the spec compiler) and a hand-written
    2X_1PORT program (4 ALU slices per element, lo in blocks 0-3, hi in 4-7),
    registered via the perf-mode table slots."""
    import copy as _copy

    import concourse.dve_ops as dve_ops
    from concourse.dve_spec import Spec, Src0, C0, Zero, One, maxx, sq, lower, Bin
    from concourse.dve_uop import (
        DveOpSpec,
        UopConfig,
        UopDpConfig,
        AluOp,
        AluInp,
        InpSel,
        OutSel,
        OutPath,
        DelayInp,
        Trigger,
        ENABLE,
    )

    existing = {op.name: op for op in dve_ops.OPS}
    if "SL1P_ANT" in existing and "SL1X_ANT" in existing:
        return existing["SL1P_ANT"], existing["SL1X_ANT"]

    body = sq(maxx(Bin(AluOp.ABSOLUTE_DIFF, Src0, C0) - One, Zero))

    def _ref_plain(in0, in1, s0, s1, imm2):
        x = in0.astype(np.float32)
        return np.square(np.maximum(np.abs(x - s0) - 1.0, 0.0)).astype(np.float32)

    def _ref_pair(in0, in1, s0, s1, imm2):
        x = in0.astype(np.float32)
        out = np.empty_like(x)
        out[:, 0] = np.square(np.maximum(np.abs(x[:, 0] - s0) - 1.0, 0.0))
        out[:, 1] = np.square(np.maximum(np.abs(x[:, 1] - s1) - 1.0, 0.0))
        return out

    base = lower(Spec(body=body, reference=_ref_plain), ver="v3")[0]

    ST, SD, NONE = (
        Trigger.SRC_TENSOR_DONE,
        Trigger.SUB_DIM_DONE,
        Trigger.NONE,
    )

    def _patch(u, const_sel, trig, nxt):
        v = _copy.deepcopy(u)
        for i in range(len(v.inp)):
            if v.inp_enable[i] and v.inp[i] == InpSel.CONST_0:
                v.inp[i] = const_sel
        v.trigger = trig
        v.next_uop = nxt
        return v

    def _mk2x(const_sel, trig, nxt):
        u = UopConfig()
        u.enable_input(InpSel.SRC_0, 1)
        u.enable_input(const_sel, 2)
        u.enable_input(InpSel.ONE_F32, 3)
        u.enable_input(InpSel.ZERO, 4)
        u.enable_input(InpSel.SRC_0_HI, 5)
        P = AluInp.PREV_ALU_OUT
        Dl = (
            AluInp.PREV_DELAY_0,
            AluInp.PREV_DELAY_1,
            AluInp.PREV_DELAY_2,
            AluInp.PREV_DELAY_3,
            AluInp.PREV_DELAY_4,
        )
        dp = u.datapath_config
        # lo element: blocks 0-3; chains: 0=src_lo 1=const 2=one 3=zero 4=src_hi
        dp[0] = (
            UopDpConfig()
            .enable_alu(AluOp.ABSOLUTE_DIFF, Dl[0], Dl[1])
            .pass_through_delay(1, 2, 3, 4)
        )
        dp[1] = (
            UopDpConfig()
            .enable_alu(AluOp.SUBTRACT, P, Dl[2])
            .pass_through_delay(1, 2, 3, 4)
        )
        dp[2] = (
            UopDpConfig()
            .enable_alu(AluOp.MAX, P, Dl[3])
            .pass_through_delay(1, 2, 3, 4)
        )
        dp[3] = (
            UopDpConfig()
            .enable_alu(AluOp.MULTIPLY, P, P)
            .pass_through_delay(1, 2, 3, 4)
        )
        # hi element: blocks 4-7; lo result rides chain 0 from block 4 on
        dp[4] = (
            UopDpConfig()
            .enable_alu(AluOp.ABSOLUTE_DIFF, Dl[4], Dl[1])
            .enable_delay_from_src(DelayInp.PREV_ALU_OUT, 0)
            .pass_through_delay(2, 3)
        )
        dp[5] = (
            UopDpConfig()
            .enable_alu(AluOp.SUBTRACT, P, Dl[2])
            .pass_through_delay(0, 3)
        )
        dp[6] = UopDpConfig().enable_alu(AluOp.MAX, P, Dl[3]).pass_through_delay(0)
        dp[7] = UopDpConfig().enable_alu(AluOp.MULTIPLY, P, P).pass_through_delay(0)
        u.enable_output(OutSel.DELAY_0, OutPath.WR0_LO)
        u.enable_output(OutSel.ALU_OUT, OutPath.WR0_HI)
        u.require_inp0 = ENABLE
        u.trigger = trig
        u.next_uop = nxt
        return u

    def _reg(name, spec, regular, uops_2x, subdim):
        row = dve_ops._CUSTOM_DVE_ROW_BASE + len(dve_ops.OPS)
        dspec = DveOpSpec(
            name=name,
            opcode=row,
            uops=regular,
            uops_2x=uops_2x,
            perf_max=1,
            rd1_en=False,
        )
        for u in regular + uops_2x:
            u.validate("v3")
        op = dve_ops.DveOp(
            name, spec, subdim=subdim, uops_sha={"v3": dspec.sha("v3")}
        )
        dve_ops.OPS.append(op)
        dve_ops._SUB_OPCODE_FOR_NAME[name] = row
        dve_ops.CUSTOM_DVE_SPECS[name] = spec
        dve_ops._COMPILE_CACHE[(name, "v3")] = dspec
        return op

    sl1p = _reg(
        "SL1P_ANT",
        Spec(body=body, reference=_ref_pair),
        [
            _patch(base, InpSel.CONST_0, (ST, SD, NONE), (0, 1, 0)),
            _patch(base, InpSel.CONST_1, (ST, SD, NONE), (0, 1, 0)),
        ],
        [
            _mk2x(InpSel.CONST_0, (ST, SD, NONE), (0, 1, 0)),
            _mk2x(InpSel.CONST_1, (ST, SD, NONE), (0, 1, 0)),
        ],
        subdim=True,
    )
    sl1x = _reg(
        "SL1X_ANT",
        Spec(body=body, reference=_ref_plain),
        [_patch(base, InpSel.CONST_0, (ST, NONE, NONE), (0, 0, 0))],
        [_mk2x(InpSel.CONST_0, (ST, NONE, NONE), (0, 0, 0))],
        subdim=False,
    )
    return sl1p, sl1x


def _path(jl):
    if jl < K:
        return "A"
    if jl >= JB - NPOOL:
        return "P"
    return "F"


def _build_nc(repeat=1):
    import concourse.bacc as bacc
    import concourse.tile as tile
    from concourse import mybir

    sl1p, sl1x = _register_custom_ops()

    dt = mybir.dt
    nc = bacc.Bacc("TRN2", target_bir_lowering=False, debug=False,
                   num_devices=NCORES)

    dram = {}
    for pfx in ("t", "s"):
        dram[pfx + "_xt"] = nc.dram_tensor(pfx + "_xt", [D, N], dt.bfloat16,
                                           kind="ExternalInput").ap()
        dram[pfx + "_xj32"] = nc.dram_tensor(pfx + "_xj32", [D, JB], dt.float32,
                                             kind="ExternalInput").ap()
        dram[pfx + "_out"] = nc.dram_tensor(pfx + "_out", [JB, N], dt.float32,
                                            kind="ExternalOutput").ap()

    with tile.TileContext(nc) as tc:
        import contextlib

        with contextlib.ExitStack() as ctx:
            singles = ctx.enter_context(tc.tile_pool(name="singles", bufs=1))
            qpool = ctx.enter_context(tc.tile_pool(name="qpool", bufs=8))
            apool = ctx.enter_context(tc.tile_pool(name="apool", bufs=4))
            vpool = ctx.enter_context(tc.tile_pool(name="vpool", bufs=4))
            ppool = ctx.enter_context(tc.tile_pool(name="ppool", bufs=3))
            opool = ctx.enter_context(tc.tile_pool(name="opool", bufs=2))
            psp = ctx.enter_context(tc.tile_pool(name="psp", bufs=2, space="PSUM"))

            # sliding -0.5 indicator for the correction reduction matmuls
            zo = singles.tile([128, 128], dt.bfloat16)
            nc.gpsimd.memset(zo, 0.0)
            nc.gpsimd.memset(zo[:, 63:64], -0.5)

            _ord = ("s", "t") if os.environ.get("SL1_SWAP", "") == "1" else ("t", "s")
            _phases = [p for _ in range(repeat) for p in _ord]

            # all input DMAs upfront, batched (multi-dim APs) and split across
            # two issue queues (SP for phase 0, Pool for phase 1) so they
            # issue and transfer in parallel from t=0
            xt_all, xj32_all, negxj_all = {}, {}, {}
            for qeng, pfx in zip((nc.sync, nc.gpsimd), _phases):
                xj32 = singles.tile([128, NT, JB], dt.float32, tag=f"{pfx}_xj32")
                qeng.dma_start(
                    out=xj32,
                    in_=dram[pfx + "_xj32"].rearrange("(t p) j -> p t j", p=128))
                # xt pairs: [128, 2, N] tiles so one paired DVE op spans two
                # partition tiles (subrow r holds d-rows 128*(2u+r)+p)
                xt = []
                for u in range(2):
                    x = singles.tile([128, 2, N], dt.bfloat16, tag=f"{pfx}_xt{u}")
                    qeng.dma_start(
                        out=x,
                        in_=dram[pfx + "_xt"][256 * u:256 * (u + 1), :]
                        .rearrange("(r p) i -> p r i", p=128))
                    xt.append(x)
                xt_all[pfx], xj32_all[pfx] = xt, xj32

            for pfx in _phases:
                # -xj in bf16: stationary for the -G matmuls (one fused op)
                negxj = singles.tile([128, NT, JB], dt.bfloat16,
                                     tag=f"{pfx}_negxj")
                nc.gpsimd.tensor_scalar(negxj, xj32_all[pfx], -1.0, None,
                                        mybir.AluOpType.mult)
                negxj_all[pfx] = negxj

            accs, outs = {}, {}
            for _pi, pfx in enumerate(_phases):
                xt, xj32, negxj = xt_all[pfx], xj32_all[pfx], negxj_all[pfx]

                def _xts(t, sl=slice(None)):
                    return xt[t // 2][:, t % 2, sl]

                acc = psp.tile([JB, N], dt.float32, tag=f"{pfx}_acc")
                accs[pfx] = acc
                for t in range(NT):
                    nc.tensor.matmul(acc, negxj[:, t, :], _xts(t), start=(t == 0),
                                     stop=False)

                # interleave the three paths so every engine has runnable work
                a_js = [j for j in range(JB) if _path(j) == "A"]
                f_js = [j for j in range(JB) if _path(j) == "F"]
                p_js = [j for j in range(JB) if _path(j) == "P"]
                order = []
                ia = if_ = ip = 0
                for _ in range(min(RUNWAY, len(f_js))):
                    order.append(f_js[if_]); if_ += 1
                INF = float("inf")
                while len(order) < JB:
                    ra = ia / len(a_js) if ia < len(a_js) else INF
                    rf = if_ / len(f_js) if if_ < len(f_js) else INF
                    rp = ip / len(p_js) if ip < len(p_js) else INF
                    m = min(ra, rf, rp)
                    if ra == m:
                        order.append(a_js[ia]); ia += 1
                    elif rf == m:
                        order.append(f_js[if_]); if_ += 1
                    else:
                        order.append(p_js[ip]); ip += 1

                # software-pipelined emission: each jl's cross-engine chain
                # (produce -> relu -> square -> matmul) is staggered by LAG
                # slots so no in-order engine queue waits at its head
                state = {}
                n_mm = [0]
                total_mm = 4 * JB

                def st_produce(jl):
                    fd = _fd(jl)
                    i0 = N - fd
                    path = _path(jl)
                    q4 = qpool.tile([128, NT, N], dt.bfloat16, tag="q4")
                    state[jl] = {"q4": q4}
                    if path == "F":
                        if SUBD:
                            for u in range(2):
                                bop = nc.vector._custom_dve(
                                    sl1p,
                                    out=q4[:, 2 * u:2 * u + 2, 0:fd],
                                    in0=xt[u][:, :, i0:N],
                                    s0=xj32[:, 2 * u, jl:jl + 1],
                                    s1=xj32[:, 2 * u + 1, jl:jl + 1])
                                if PERF:
                                    bop.ins.perf_max = 1
                        else:
                            for t in range(NT):
                                bop = nc.vector._custom_dve(
                                    sl1x,
                                    out=q4[:, t, 0:fd],
                                    in0=_xts(t, slice(i0, N)),
                                    s0=xj32[:, t, jl:jl + 1])
                                if PERF:
                                    bop.ins.perf_max = 1
                    elif path == "A":
                        a4 = apool.tile([128, NT, N], dt.bfloat16, tag="a4")
                        for t in range(NT):
                            nc.scalar.activation(a4[:, t, 0:fd],
                                                 _xts(t, slice(i0, N)),
                                                 mybir.ActivationFunctionType.Abs,
                                                 bias=xj32[:, t, jl:jl + 1],
                                                 scale=-1.0)
                        state[jl]["a4"] = a4
                    else:  # "P"
                        p4 = ppool.tile([128, NT, N], dt.bfloat16, tag="p4")
                        for t in range(NT):
                            nc.gpsimd.tensor_scalar(p4[:, t, 0:fd],
                                                    _xts(t, slice(i0, N)),
                                                    xj32[:, t, jl:jl + 1], 0.0,
                                                    mybir.AluOpType.subtract,
                                                    mybir.AluOpType.abs_max)
                        v4 = vpool.tile([128, NT, N], dt.bfloat16, tag="v4")
                        nc.gpsimd.tensor_scalar(v4[:, :, 0:fd], p4[:, :, 0:fd],
                                                1.0, 0.0,
                                                mybir.AluOpType.subtract,
                                                mybir.AluOpType.max)
                        nc.gpsimd.tensor_tensor(q4[:, :, 0:fd], v4[:, :, 0:fd],
                                                v4[:, :, 0:fd],
                                                mybir.AluOpType.mult)

                def st_relu(jl):
                    if _path(jl) != "A":
                        return
                    fd = _fd(jl)
                    v4 = vpool.tile([128, NT, N], dt.bfloat16, tag="v4")
                    a4 = state[jl]["a4"]
                    veng = nc.gpsimd if jl >= K - VP else nc.vector
                    veng.tensor_scalar(v4[:, :, 0:fd], a4[:, :, 0:fd],
                                       1.0, 0.0, mybir.AluOpType.subtract,
                                       mybir.AluOpType.max)
                    state[jl]["v4"] = v4

                def st_square(jl):
                    if _path(jl) != "A":
                        return
                    fd = _fd(jl)
                    v4 = state[jl]["v4"]
                    q4 = state[jl]["q4"]
                    if jl < NSQA:
                        nc.scalar.activation(q4[:, :, 0:fd], v4[:, :, 0:fd],
                                             mybir.ActivationFunctionType.Square,
                                             bias=0.0, scale=1.0)
                    else:
                        sqeng = nc.gpsimd if jl >= K - NSQP else nc.vector
                        sqeng.tensor_tensor(q4[:, :, 0:fd], v4[:, :, 0:fd],
                                            v4[:, :, 0:fd], mybir.AluOpType.mult)

                def st_matmul(jl):
                    fd = _fd(jl)
                    i0 = N - fd
                    q4 = state[jl]["q4"]
                    for t in range(NT):
                        n_mm[0] += 1
                        nc.tensor.matmul(acc[:, i0:N], zo[:, 63 - jl:127 - jl],
                                         q4[:, t, 0:fd],
                                         start=False, stop=(n_mm[0] == total_mm))
                    del state[jl]

                stages = (st_produce, st_relu, st_square, st_matmul)
                lags = (0, 1, 2, LAG)
                for slot in range(len(order) + LAG):
                    for stage, lag in zip(stages, lags):
                        i = slot - lag
                        if 0 <= i < len(order):
                            stage(order[i])

            # copies + output DMAs at the very end: nothing queues behind them
            for pfx in _phases:
                out_sb = opool.tile([JB, N], dt.float32, tag=f"{pfx}_out")
                if COPY_ENG == "pool":
                    nc.gpsimd.tensor_copy(out_sb, accs[pfx])
                elif COPY_ENG == "dve":
                    nc.vector.tensor_copy(out_sb, accs[pfx])
                else:
                    nc.scalar.copy(out_sb, accs[pfx])
                nc.sync.dma_start(out=dram[pfx + "_out"], in_=out_sb)

    nc.finalize()
    return nc


def _get_nc(repeat=1):
    key = ("nc", repeat)
    if key not in _CACHE:
        _CACHE[key] = _build_nc(repeat=repeat)
    return _CACHE[key]


def _prep_inputs(teacher, student):
    in_maps = []
    prepped = {}
    for pfx, x in (("t", teacher), ("s", student)):
        xb = np.asarray(x, np.float32).astype(ml_dtypes.bfloat16)   # [N, D] bf16
        xtb = np.ascontiguousarray(xb.T)                            # [D, N] bf16
        xtb32 = xtb.astype(np.float32)  # bf16-rounded values, exact in fp32
        prepped[pfx] = (xtb, xtb32)
    for k in range(NCORES):
        m = {}
        for pfx in ("t", "s"):
            xtb, xtb32 = prepped[pfx]
            m[pfx + "_xt"] = xtb
            m[pfx + "_xj32"] = np.ascontiguousarray(xtb32[:, k::8])
        in_maps.append(m)
    return in_maps


def _assemble(blocks, n):
    """blocks: list of [JB, N] device rows (-G + corr) per core; n: [N] fp64
    squared-norm vector. Returns the full symmetric pair-sum matrix [N, N]."""
    U = np.zeros((N, N), np.float64)
    for k in range(NCORES):
        b = blocks[k].astype(np.float64)
        for jl in range(JB):
            j = 8 * jl + k
            U[j, j + 1:] = b[jl, j + 1:]
    T = U + U.T
    M = 0.5 * (n[:, None] + n[None, :])
    np.fill_diagonal(M, 0.0)
    T += M
    np.fill_diagonal(T, 0.0)
    return T


def run_device(teacher, student, **kwargs):
    """Run the device part; returns (T, S) full pair-sum matrices and results."""
    from concourse.bass_utils import run_bass_kernel_spmd

    nc = _get_nc()
    in_maps = _prep_inputs(teacher, student)
    res = run_bass_kernel_spmd(nc, in_maps, core_ids=list(range(NCORES)), **kwargs)
    ns = {}
    for pfx, x in (("t", teacher), ("s", student)):
        xb32 = np.asarray(x, np.float32).astype(ml_dtypes.bfloat16).astype(np.float64)
        ns[pfx] = np.square(xb32).sum(axis=1)
    T = _assemble([res.results[k]["t_out"] for k in range(NCORES)], ns["t"])
    S = _assemble([res.results[k]["s_out"] for k in range(NCORES)], ns["s"])
    return T, S, res


def kernel(teacher, student):
    teacher = np.asarray(teacher)
    student = np.asarray(student)
    T, S, _ = run_device(teacher, student)
    out = np.abs(T / T.mean() - S / S.mean()).sum()
    return np.float32(out)


if __name__ == "__main__":
    rng = np.random.default_rng(0)
    t = rng.standard_normal((N, D)).astype(np.float32)
    s = rng.standard_normal((N, D)).astype(np.float32)
    print(kernel(t, s))
